# revision 21
# baseline (speedup 1.0000x reference)
"""Trainium2 Bass kernel for nn_CfaModel (retrieval_knn).

Computes, for features [16, 3136, 1792], memory_bank [1792, 3136], radius [1]:
    distance[b,n,k] = ||f[b,n]||^2 + ||c[k]||^2 - 2 f.c
    vals = 6 smallest distances per (b,n)  (ascending)
    l_att = (1/NU) * mean(relu(vals[..., :3] - r^2))
    l_rep = (1/NU) * mean(relu(r^2 - vals[..., 3:] - ALPHA))
    out   = l_att + l_rep   (scalar, float32)

Strategy: data-parallel over batch across 8 NeuronCores (2 samples each).
Per core, stream 128-row tiles of pre-transposed features (fp8); TensorE
computes mm = 2 f.c via fp8 DoubleRow matmuls (2 contraction chunks per
pass, PE at peak fp8 rate); VectorE subtracts the (host-precomputed)
bf16 residual of ||c||^2 and extracts the 8 largest per row with max8
(= 8 smallest distances); ScalarE folds each tile's top-k into the
att/rep relu partial sums using host-precomputed per-partition biases
(fsq + mean_c_sq -/+ r^2 terms).  Host sums the per-core partials.

All row/column norms are computed on the host (they are O(N*C) vs the
O(N*C*K) cross term) so the device does nothing but the matmul, the
top-k scan, and the loss reduction.
"""

import os
import threading

import numpy as np
import ml_dtypes

import concourse.bass as bass
import concourse.mybir as mybir
import concourse.tile as tile
from concourse import bacc
import concourse.bass_utils as bass_utils
from concourse.bass_utils import run_bass_kernel_spmd

# Problem constants (hardcoded per the harness contract).
B, HW, C, K = 16, 3136, 1792, 3136
NU, ALPHA = 0.001, 0.1
NCORES = 8
BPC = B // NCORES          # batches per core = 2
ROWS = BPC * HW            # rows per core = 6272
P = 128                    # partitions
NT = ROWS // P             # row tiles per core = 49
KC = C // P                # contraction chunks = 14
NQ = KC // 2               # DoubleRow passes per column tile = 7
CT = 7                     # column tiles
CW = K // CT               # column tile width = 448
KH = 6                     # m2 chunk split point between the two DMA queues
NHEAD = 3                  # head tiles interleaved over column blocks

FP32 = mybir.dt.float32
BF16 = mybir.dt.bfloat16
FP8 = mybir.dt.float8e4
AF = mybir.ActivationFunctionType


def build_module(nt=NT):
    nc = bacc.Bacc(trn_type="TRN2", target_bir_lowering=False)

    # pre-transposed f (c on partitions) as the matmul stationary operand
    fT_dram = nc.dram_tensor("fT", [nt, P, KC, P], FP8, kind="ExternalInput")
    # m2 in column-block-major layout so each block DMA is one contiguous
    # run per partition (few fat descriptors -> cheap trigger, fast load)
    m2_dram = nc.dram_tensor("m2", [CT, P, KC, CW], FP8, kind="ExternalInput")
    # c_sq residual (c_sq - mean), bf16, broadcast to all partitions
    cr_dram = nc.dram_tensor("cres", [1, K], BF16, kind="ExternalInput")
    # per-(partition, tile) loss biases: attb = fsq + mu_c - r^2,
    # repb = r^2 - ALPHA - fsq - mu_c
    ab_dram = nc.dram_tensor("attb", [P, nt], FP32, kind="ExternalInput")
    rb_dram = nc.dram_tensor("repb", [P, nt], FP32, kind="ExternalInput")
    out_dram = nc.dram_tensor("out", [P, 2, nt], FP32, kind="ExternalOutput")

    with tile.TileContext(nc) as tc:
        with tc.tile_pool(name="singles", bufs=1) as singles:
            # ---- persistent tiles ----
            m2 = singles.tile([P, CT, KC, CW], FP8)  # 2*memory_bank, C on partitions
            cres = singles.tile([P, K], BF16)      # c_sq residual, bcast on partitions
            attb = singles.tile([P, nt], FP32)
            repb = singles.tile([P, nt], FP32)
            g8 = singles.tile([P, nt, 8], FP32)    # top-8 of 2f.c - cres per tile
            parts = singles.tile([P, 2, nt], FP32)  # [:,0,:] att, [:,1,:] rep
            att_scr = singles.tile([P, 3], FP32)   # ScalarE relu scratch
            rep_scr = singles.tile([P, 3], FP32)
            # last two tiles: per-block top-8 scratch (keeps the final
            # VectorE scan off the post-matmul critical path)
            g8b = singles.tile([P, 2, CT, 8], FP32)

            with (
                tc.tile_pool(name="ftp", bufs=NHEAD + 1) as ftp,
                tc.tile_pool(name="gpp", bufs=NHEAD) as gpp,
                tc.tile_pool(name="mmp", bufs=8, space="PSUM") as mmp,
            ):
                def load_ft(t, eng=None):
                    fT_t = ftp.tile([P, KC, P], FP8, name="fT")
                    (eng or nc.gpsimd).dma_start(fT_t[:], fT_dram[t])
                    return fT_t

                # ---- DMA schedule ----
                # Supply-ordered: the first matmul (tile 0, block 0, q=0)
                # needs only m2[block0, chunks 0:2] + fT0[chunks 0:2], so
                # those tiny pieces go first on the sync queue; the rest of
                # block 0 follows, split across both queues.  Then blocks
                # 1..6 stream in block order (sync: chunks 0:KH, gpsimd:
                # KH:), matching the column-block order in which the head
                # tiles (interleaved over blocks below) consume them.
                fT0 = ftp.tile([P, KC, P], FP8, name="fT")
                nc.sync.dma_start(m2[:, 0, 0:2, :], m2_dram[0, :, 0:2, :])
                nc.sync.dma_start(fT0[:, 0:2, :], fT_dram[0, :, 0:2, :])
                nc.sync.dma_start(fT0[:, 2:, :], fT_dram[0, :, 2:, :])
                nc.sync.dma_start(m2[:, 0, 2:KH, :], m2_dram[0, :, 2:KH, :])
                fts = [fT0, load_ft(1)]
                nc.gpsimd.dma_start(m2[:, 0, KH:, :], m2_dram[0, :, KH:, :])
                fts.append(load_ft(2))
                for j in range(1, CT):
                    nc.sync.dma_start(m2[:, j, :KH, :], m2_dram[j, :, :KH, :])
                    nc.gpsimd.dma_start(m2[:, j, KH:, :], m2_dram[j, :, KH:, :])
                nc.sync.dma_start(cres[:], cr_dram[:].to_broadcast([P, K]))
                nc.gpsimd.dma_start(attb[:], ab_dram[:])
                nc.gpsimd.dma_start(repb[:], rb_dram[:])

                def emit_group(fT_t, gp, t, j, tail=-1):
                    # one accumulation group: 2 f.c for column block j,
                    # then VectorE applies the c_sq residual into gp
                    cs = slice(j * CW, (j + 1) * CW)
                    mm = mmp.tile([P, CW], FP32, name="acc")
                    for q in range(NQ):
                        nc.tensor.matmul(
                            mm[:],
                            fT_t[:, 2 * q:2 * q + 2, :],
                            m2[:, j, 2 * q:2 * q + 2, :],
                            start=(q == 0),
                            stop=(q == NQ - 1),
                            perf_mode=mybir.MatmulPerfMode.DoubleRow,
                        )
                    nc.vector.tensor_sub(gp[:, cs], mm[:], cres[:, cs])
                    if tail >= 0:
                        # final row tiles: per-block top-8 so the scan
                        # overlaps the remaining matmuls (short tail)
                        nc.vector.max(out=g8b[:, tail, j, :], in_=gp[:, cs])

                def emit_reduce(gp, t, tail=-1):
                    # top-8 largest (descending) == 8 smallest distances;
                    # then fold this tile into the loss partial sums
                    if tail >= 0:
                        nc.vector.max(out=g8[:, t, :], in_=g8b[:, tail, 0:CT, :])
                    else:
                        nc.vector.max(out=g8[:, t, :], in_=gp[:])
                    # att = relu(dist - r^2) = relu(-gp_sel + (fsq + mu_c - r^2))
                    nc.scalar.activation(
                        att_scr[:], g8[:, t, 0:3], AF.Relu,
                        bias=attb[:, t:t + 1], scale=-1.0,
                        accum_out=parts[:, 0, t:t + 1],
                    )
                    # rep = relu(r^2 - dist - a) = relu(gp_sel + (r^2 - a - fsq - mu_c))
                    nc.scalar.activation(
                        rep_scr[:], g8[:, t, 3:6], AF.Relu,
                        bias=repb[:, t:t + 1], scale=1.0,
                        accum_out=parts[:, 1, t:t + 1],
                    )

                # ---- tiles 0..NHEAD-1, interleaved over column blocks ----
                gps = [gpp.tile([P, K], FP32, name="gp") for _ in range(NHEAD)]
                for j in range(CT):
                    if j == 1:
                        fts.append(load_ft(NHEAD))
                    for th in range(NHEAD):
                        emit_group(fts[th], gps[th], th, j)
                for th in range(NHEAD):
                    emit_reduce(gps[th], th)

                # ---- steady-state tiles ----
                for t in range(NHEAD, nt):
                    tail = t - (nt - 2)  # -> 0, 1 for the last two tiles
                    if t + 1 < nt:
                        fts.append(load_ft(t + 1))
                    gp = gpp.tile([P, K], FP32, name="gp")
                    for j in range(CT):
                        emit_group(fts[t], gp, t, j, tail=tail)
                    emit_reduce(gp, t, tail=tail)

            # out DMA triggered from the (otherwise idle) scalar queue, which
            # is also the producer of the partial sums -> no cross-engine hop
            nc.scalar.dma_start(out_dram[:], parts[:])

    nc.compile()
    return nc


_CACHE = {}
_LOCK = threading.Lock()
LAST_RESULT = None


def _get_module(nt=NT):
    with _LOCK:
        if nt not in _CACHE:
            _CACHE[nt] = build_module(nt)
        return _CACHE[nt]


def prep_inputs(features, memory_bank, radius):
    # pre-transposed matmul operand: [core, t, p(=c%128), ci, r]
    fT = np.ascontiguousarray(
        features.reshape(NCORES, NT, P, KC, P).transpose(0, 1, 4, 3, 2)
    ).astype(ml_dtypes.float8_e4m3)
    # column-block-major: [CT, P(=c%128), KC, CW]
    m2 = (
        (2.0 * memory_bank)
        .reshape(KC, P, CT, CW)
        .transpose(2, 1, 0, 3)
        .astype(ml_dtypes.float8_e4m3)
        .copy()
    )

    # host-side norms: O(N*C) work vs the device's O(N*C*K)
    c_sq = np.einsum("ck,ck->k", memory_bank, memory_bank, dtype=np.float64)
    mu_c = float(c_sq.mean())
    cres = (c_sq - mu_c).astype(np.float32).reshape(1, K)
    cres_bf = cres.astype(ml_dtypes.bfloat16)

    fsq = np.einsum(
        "btpc,btpc->btp",
        features.reshape(NCORES, NT, P, C),
        features.reshape(NCORES, NT, P, C),
        dtype=np.float64,
    )  # [core, t, p]
    r2 = float(np.float64(radius.reshape(-1)[0]) ** 2)
    attb = (fsq + mu_c - r2).transpose(0, 2, 1).astype(np.float32)   # [core, p, t]
    repb = (r2 - ALPHA - fsq - mu_c).transpose(0, 2, 1).astype(np.float32)
    return fT, m2, cres_bf, attb, repb


def kernel(features, memory_bank, radius):
    global LAST_RESULT
    features = np.asarray(features, dtype=np.float32)
    memory_bank = np.asarray(memory_bank, dtype=np.float32)
    radius = np.asarray(radius, dtype=np.float32)
    assert features.shape == (B, HW, C)
    assert memory_bank.shape == (C, K)

    nc = _get_module()

    # Shard: batch-parallel, 2 samples per core.  Low-precision cast on
    # host; distances accumulate fp32 on device.
    fT, m2, cres_bf, attb, repb = prep_inputs(features, memory_bank, radius)

    in_maps = [
        {"fT": fT[i], "m2": m2, "cres": cres_bf, "attb": attb[i],
         "repb": repb[i]}
        for i in range(NCORES)
    ]
    trace = bool(int(os.environ.get("KNN_TRACE", "0")))
    try:
        res = run_bass_kernel_spmd(
            nc, in_maps, core_ids=list(range(NCORES)), trace=trace
        )
    except ModuleNotFoundError:
        # axon NTFF profiling hook unavailable in this environment
        res = run_bass_kernel_spmd(
            nc, in_maps, core_ids=list(range(NCORES)), trace=False
        )
    LAST_RESULT = res

    parts = np.stack([r["out"] for r in res.results])   # [8, 2, 128, nt]
    total = parts.sum(dtype=np.float64)
    cnt = B * HW * 3
    loss = total / cnt / NU
    return np.float32(loss)


# revision 22
# speedup vs baseline: 1.0140x; 1.0140x over previous
"""Trainium2 Bass kernel for nn_CfaModel (retrieval_knn).

Computes, for features [16, 3136, 1792], memory_bank [1792, 3136], radius [1]:
    distance[b,n,k] = ||f[b,n]||^2 + ||c[k]||^2 - 2 f.c
    vals = 6 smallest distances per (b,n)  (ascending)
    l_att = (1/NU) * mean(relu(vals[..., :3] - r^2))
    l_rep = (1/NU) * mean(relu(r^2 - vals[..., 3:] - ALPHA))
    out   = l_att + l_rep   (scalar, float32)

Strategy: data-parallel over batch across 8 NeuronCores (2 samples each).
Per core, stream 128-row tiles of pre-transposed features (fp8); TensorE
computes mm = 2 f.c via fp8 DoubleRow matmuls (2 contraction chunks per
pass, PE at peak fp8 rate); VectorE subtracts the (host-precomputed)
bf16 residual of ||c||^2 and extracts the 8 largest per row with max8
(= 8 smallest distances); ScalarE folds each tile's top-k into the
att/rep relu partial sums using host-precomputed per-partition biases
(fsq + mean_c_sq -/+ r^2 terms).  Host sums the per-core partials.

All row/column norms are computed on the host (they are O(N*C) vs the
O(N*C*K) cross term) so the device does nothing but the matmul, the
top-k scan, and the loss reduction.
"""

import os
import threading

import numpy as np
import ml_dtypes

import concourse.bass as bass
import concourse.mybir as mybir
import concourse.tile as tile
from concourse import bacc
import concourse.bass_utils as bass_utils
from concourse.bass_utils import run_bass_kernel_spmd

# Problem constants (hardcoded per the harness contract).
B, HW, C, K = 16, 3136, 1792, 3136
NU, ALPHA = 0.001, 0.1
NCORES = 8
BPC = B // NCORES          # batches per core = 2
ROWS = BPC * HW            # rows per core = 6272
P = 128                    # partitions
NT = ROWS // P             # row tiles per core = 49
KC = C // P                # contraction chunks = 14
NQ = KC // 2               # DoubleRow passes per column tile = 7
CT = 7                     # column tiles
CW = K // CT               # column tile width = 448
KH = 6                     # m2 chunk split point between the two DMA queues
NHEAD = 3                  # head tiles interleaved over column blocks

FP32 = mybir.dt.float32
BF16 = mybir.dt.bfloat16
FP8 = mybir.dt.float8e4
AF = mybir.ActivationFunctionType


def build_module(nt=NT):
    nc = bacc.Bacc(trn_type="TRN2", target_bir_lowering=False)

    # pre-transposed f (c on partitions) as the matmul stationary operand
    fT_dram = nc.dram_tensor("fT", [nt, P, KC, P], FP8, kind="ExternalInput")
    # m2 in column-block-major layout so each block DMA is one contiguous
    # run per partition (few fat descriptors -> cheap trigger, fast load)
    m2_dram = nc.dram_tensor("m2", [CT, P, KC, CW], FP8, kind="ExternalInput")
    # c_sq residual (c_sq - mean), bf16, broadcast to all partitions
    cr_dram = nc.dram_tensor("cres", [1, K], BF16, kind="ExternalInput")
    # per-(partition, tile) loss biases: attb = fsq + mu_c - r^2,
    # repb = r^2 - ALPHA - fsq - mu_c
    ab_dram = nc.dram_tensor("attb", [P, nt], FP32, kind="ExternalInput")
    rb_dram = nc.dram_tensor("repb", [P, nt], FP32, kind="ExternalInput")
    out_dram = nc.dram_tensor("out", [P, 2, nt], FP32, kind="ExternalOutput")

    with tile.TileContext(nc) as tc:
        with tc.tile_pool(name="singles", bufs=1) as singles:
            # ---- persistent tiles ----
            m2 = singles.tile([P, CT, KC, CW], FP8)  # 2*memory_bank, C on partitions
            cres = singles.tile([P, K], BF16)      # c_sq residual, bcast on partitions
            attb = singles.tile([P, nt], FP32)
            repb = singles.tile([P, nt], FP32)
            g8 = singles.tile([P, nt, 8], FP32)    # top-8 of 2f.c - cres per tile
            parts = singles.tile([P, 2, nt], FP32)  # [:,0,:] att, [:,1,:] rep
            att_scr = singles.tile([P, 3], FP32)   # ScalarE relu scratch
            rep_scr = singles.tile([P, 3], FP32)
            # last two tiles: per-block top-8 scratch (keeps the final
            # VectorE scan off the post-matmul critical path)
            g8b = singles.tile([P, 2, CT, 8], FP32)

            with (
                tc.tile_pool(name="ftp", bufs=NHEAD + 1) as ftp,
                tc.tile_pool(name="gpp", bufs=NHEAD) as gpp,
                tc.tile_pool(name="mmp", bufs=8, space="PSUM") as mmp,
            ):
                def load_ft(t, eng=None):
                    fT_t = ftp.tile([P, KC, P], FP8, name="fT")
                    (eng or nc.gpsimd).dma_start(fT_t[:], fT_dram[t])
                    return fT_t

                # ---- DMA schedule ----
                # Supply-ordered: the first matmul (tile 0, block 0, q=0)
                # needs only m2[block0, chunks 0:2] + fT0[chunks 0:2], so
                # those tiny pieces go first on the sync queue; the rest of
                # block 0 follows, split across both queues.  Then blocks
                # 1..6 stream in block order (sync: chunks 0:KH, gpsimd:
                # KH:), matching the column-block order in which the head
                # tiles (interleaved over blocks below) consume them.
                fT0 = ftp.tile([P, KC, P], FP8, name="fT")
                nc.sync.dma_start(m2[:, 0, 0:2, :], m2_dram[0, :, 0:2, :])
                nc.sync.dma_start(fT0[:, 0:2, :], fT_dram[0, :, 0:2, :])
                nc.sync.dma_start(fT0[:, 2:, :], fT_dram[0, :, 2:, :])
                nc.sync.dma_start(m2[:, 0, 2:KH, :], m2_dram[0, :, 2:KH, :])
                nc.gpsimd.dma_start(m2[:, 0, KH:, :], m2_dram[0, :, KH:, :])
                fts = [fT0, load_ft(1), load_ft(2)]

                def load_cres(j):
                    cs = slice(j * CW, (j + 1) * CW)
                    nc.sync.dma_start(
                        cres[:, cs], cr_dram[:, cs].to_broadcast([P, CW])
                    )

                load_cres(0)
                for j in range(1, CT):
                    nc.sync.dma_start(m2[:, j, :KH, :], m2_dram[j, :, :KH, :])
                    nc.gpsimd.dma_start(m2[:, j, KH:, :], m2_dram[j, :, KH:, :])
                    load_cres(j)
                nc.gpsimd.dma_start(attb[:], ab_dram[:])
                nc.gpsimd.dma_start(repb[:], rb_dram[:])

                def emit_group(fT_t, gp, t, j, tail=-1):
                    # one accumulation group: 2 f.c for column block j,
                    # then VectorE applies the c_sq residual into gp
                    cs = slice(j * CW, (j + 1) * CW)
                    mm = mmp.tile([P, CW], FP32, name="acc")
                    for q in range(NQ):
                        nc.tensor.matmul(
                            mm[:],
                            fT_t[:, 2 * q:2 * q + 2, :],
                            m2[:, j, 2 * q:2 * q + 2, :],
                            start=(q == 0),
                            stop=(q == NQ - 1),
                            perf_mode=mybir.MatmulPerfMode.DoubleRow,
                        )
                    nc.vector.tensor_sub(gp[:, cs], mm[:], cres[:, cs])
                    if tail >= 0:
                        # final row tiles: per-block top-8 so the scan
                        # overlaps the remaining matmuls (short tail)
                        nc.vector.max(out=g8b[:, tail, j, :], in_=gp[:, cs])

                def emit_reduce(gp, t, tail=-1):
                    # top-8 largest (descending) == 8 smallest distances;
                    # then fold this tile into the loss partial sums
                    if tail >= 0:
                        nc.vector.max(out=g8[:, t, :], in_=g8b[:, tail, 0:CT, :])
                    else:
                        nc.vector.max(out=g8[:, t, :], in_=gp[:])
                    # att = relu(dist - r^2) = relu(-gp_sel + (fsq + mu_c - r^2))
                    nc.scalar.activation(
                        att_scr[:], g8[:, t, 0:3], AF.Relu,
                        bias=attb[:, t:t + 1], scale=-1.0,
                        accum_out=parts[:, 0, t:t + 1],
                    )
                    # rep = relu(r^2 - dist - a) = relu(gp_sel + (r^2 - a - fsq - mu_c))
                    nc.scalar.activation(
                        rep_scr[:], g8[:, t, 3:6], AF.Relu,
                        bias=repb[:, t:t + 1], scale=1.0,
                        accum_out=parts[:, 1, t:t + 1],
                    )

                # ---- tiles 0..NHEAD-1, interleaved over column blocks ----
                gps = [gpp.tile([P, K], FP32, name="gp") for _ in range(NHEAD)]
                for j in range(CT):
                    if j == 1:
                        fts.append(load_ft(NHEAD))
                    for th in range(NHEAD):
                        emit_group(fts[th], gps[th], th, j)
                for th in range(NHEAD):
                    emit_reduce(gps[th], th)

                # ---- steady-state tiles ----
                for t in range(NHEAD, nt):
                    tail = t - (nt - 2)  # -> 0, 1 for the last two tiles
                    if t + 1 < nt:
                        fts.append(load_ft(t + 1))
                    gp = gpp.tile([P, K], FP32, name="gp")
                    for j in range(CT):
                        emit_group(fts[t], gp, t, j, tail=tail)
                    emit_reduce(gp, t, tail=tail)

            # out DMA triggered from the (otherwise idle) scalar queue, which
            # is also the producer of the partial sums -> no cross-engine hop
            nc.scalar.dma_start(out_dram[:], parts[:])

    nc.compile()
    return nc


_CACHE = {}
_LOCK = threading.Lock()
LAST_RESULT = None


def _get_module(nt=NT):
    with _LOCK:
        if nt not in _CACHE:
            _CACHE[nt] = build_module(nt)
        return _CACHE[nt]


def prep_inputs(features, memory_bank, radius):
    # pre-transposed matmul operand: [core, t, p(=c%128), ci, r]
    fT = np.ascontiguousarray(
        features.reshape(NCORES, NT, P, KC, P).transpose(0, 1, 4, 3, 2)
    ).astype(ml_dtypes.float8_e4m3)
    # column-block-major: [CT, P(=c%128), KC, CW]
    m2 = (
        (2.0 * memory_bank)
        .reshape(KC, P, CT, CW)
        .transpose(2, 1, 0, 3)
        .astype(ml_dtypes.float8_e4m3)
        .copy()
    )

    # host-side norms: O(N*C) work vs the device's O(N*C*K)
    c_sq = np.einsum("ck,ck->k", memory_bank, memory_bank, dtype=np.float64)
    mu_c = float(c_sq.mean())
    cres = (c_sq - mu_c).astype(np.float32).reshape(1, K)
    cres_bf = cres.astype(ml_dtypes.bfloat16)

    fsq = np.einsum(
        "btpc,btpc->btp",
        features.reshape(NCORES, NT, P, C),
        features.reshape(NCORES, NT, P, C),
        dtype=np.float64,
    )  # [core, t, p]
    r2 = float(np.float64(radius.reshape(-1)[0]) ** 2)
    attb = (fsq + mu_c - r2).transpose(0, 2, 1).astype(np.float32)   # [core, p, t]
    repb = (r2 - ALPHA - fsq - mu_c).transpose(0, 2, 1).astype(np.float32)
    return fT, m2, cres_bf, attb, repb


def kernel(features, memory_bank, radius):
    global LAST_RESULT
    features = np.asarray(features, dtype=np.float32)
    memory_bank = np.asarray(memory_bank, dtype=np.float32)
    radius = np.asarray(radius, dtype=np.float32)
    assert features.shape == (B, HW, C)
    assert memory_bank.shape == (C, K)

    nc = _get_module()

    # Shard: batch-parallel, 2 samples per core.  Low-precision cast on
    # host; distances accumulate fp32 on device.
    fT, m2, cres_bf, attb, repb = prep_inputs(features, memory_bank, radius)

    in_maps = [
        {"fT": fT[i], "m2": m2, "cres": cres_bf, "attb": attb[i],
         "repb": repb[i]}
        for i in range(NCORES)
    ]
    trace = bool(int(os.environ.get("KNN_TRACE", "0")))
    try:
        res = run_bass_kernel_spmd(
            nc, in_maps, core_ids=list(range(NCORES)), trace=trace
        )
    except ModuleNotFoundError:
        # axon NTFF profiling hook unavailable in this environment
        res = run_bass_kernel_spmd(
            nc, in_maps, core_ids=list(range(NCORES)), trace=False
        )
    LAST_RESULT = res

    parts = np.stack([r["out"] for r in res.results])   # [8, 2, 128, nt]
    total = parts.sum(dtype=np.float64)
    cnt = B * HW * 3
    loss = total / cnt / NU
    return np.float32(loss)
